# revision 6
# baseline (speedup 1.0000x reference)
"""Trainium2 Bass kernel for the ADMM total-variation solver (nn_ADMM).

Math: x <- B^-1(bA + v) iterated 50x, B = AtA + g*DtD + a*I.  AtA is
rank-9 and C := g*DtD + a*I is circulant, so by Woodbury
    B^-1 = C^-1 - W2 U^T,   U = C^-1 A^T,  W2 = U S^-1,  S = I9 + A U.
C^-1 is applied as a banded (radius-32) circular convolution G; the
rank-9 correction uses q = U^T v.  All 8 cores run the same program
(SPMD, no collectives); core 0's output is returned.

v3 design (vs the En/P1/P2n baseline):
- state algebra: tAB := eta - Bx (f32), tTW := tau - alpha*x (f32).
  bankAt = AB x + tAB, bankB = B x + tAB (= eta), tAB' = bankB - Un,
  tTW' = u3 - Wn =: T3, v = -T3 - tAB'.
- tAB' is written by DVE DIRECTLY into the two PSUM banks that the next
  iteration's PE matmuls accumulate onto (start=False) - no I-fold
  matmuls, no bf16 round of the eta state, no En add.
- the tau-side chain (u3 = a*x + tTW, Dt = a*x + u3, Wn = relu Dt,
  T3 = u3 - Wn) runs entirely on GpSimd; the Scalar engine is unused.
- both grouped rank-9 reductions are fused mul+segmented-scan:
  tensor_tensor_scan(state = mask*state + prod) leaves each group's
  total at its last element; the ones-matmul / Xn read the tails via
  strided APs.  q stays in PSUM f32 (no bf16 cast round-trip).
- G = hi+lo bf16 split for the main band; corner (cross-column) pieces
  are hi-only (lo corners dropped; ~2e-4 operator effect).

Vector layout: [128, 32] tiles, flat index i = k + 128*c at tile col c.
"""

import numpy as np

N = 4096
P = 128          # partitions
CCOL = 32        # payload columns; i = k + 128*c
RB = 32          # band radius of G
R9 = 9           # Woodbury rank
R10 = 10         # rank columns incl. the c0 slot
GAMMA = 10.0
ALPHA = 5.0
LAM = 1e-4
NIT = 50
NCORES = 8
USE_SCAN = True   # fused mul+segmented-scan reduces (False: tensor_reduce)

# f32-column offsets inside the constant blob [128, BLOB_COLS].
# bf16 payloads are packed two-per-f32-column and bitcast on device.
_cur = 0
def _alloc(w):
    global _cur
    off = _cur
    _cur += w
    return off

OFF_A1M   = _alloc(64)    # gamma*(S+ - I) main, bf16 [128,128] (iter 1)
OFF_BM    = _alloc(64)    # gamma*(S- - I) main
OFF_ABM   = _alloc(64)    # gamma*(S+ + S- - 2I) main (A1_M + B_M)
OFF_A1C   = _alloc(64)    # A1 corner (single element, padded)
OFF_BC    = _alloc(64)    # B corner
OFF_GHIM  = _alloc(64)    # C^-1 banded main, hi half
OFF_GLOM  = _alloc(64)    # lo half
OFF_GBLH  = _alloc(64)    # C^-1 left corner hi
OFF_GBHH  = _alloc(64)    # C^-1 right corner hi
OFF_ONES  = _alloc(64)    # all-ones bf16 (partition reduce + broadcast)
OFF_U2    = _alloc(144)   # U m-major: [k, m*32+c], bf16 [128, 288]
OFF_W2    = _alloc(160)   # [W2 | c0] c-major: [k, c*10+m], bf16
OFF_X0    = _alloc(16)    # x0 tile, bf16 [128, 32]
OFF_M288  = _alloc(144)   # scan reset mask for Z1: 0 at c==0, else 1
OFF_M320  = _alloc(160)   # scan reset mask for Z2: 0 at m==0, else 1
OFF_NEG   = _alloc(1)     # [-1/128, 0] bf16 pair (c0 slot of bankR)
BLOB_COLS = _cur


def _bf16(x):
    x32 = np.asarray(x, np.float32)
    u = x32.view(np.uint32)
    r = ((u >> 16) + ((u >> 15) & 1)).astype(np.uint32) << 16
    return r.view(np.float32)


def _pack_bf16(arr):
    """[128, W] float (W even) -> [128, W//2] f32 with packed bf16 pairs."""
    a = _bf16(arr).view(np.uint32) >> 16
    lo, hi = a[:, 0::2], a[:, 1::2]
    return (lo | (hi << 16)).view(np.float32)


def _banded(h):
    """main/BL/BH lhsT pieces for kernel h (dict d -> coef), [128,128] each.
    lhsT[k, m]: contraction index k = input row, m = output row.
    main: within-column (shift d = k - m);
    BL: rhs = col c-1 view (shift d = k - 128 - m);
    BH: rhs = col c+1 view (shift d = k + 128 - m)."""
    B0 = np.zeros((P, P)); BL = np.zeros((P, P)); BH = np.zeros((P, P))
    for k in range(P):
        for m in range(P):
            if (k - m) in h:
                B0[k, m] = h[k - m]
            if (k - P - m) in h:
                BL[k, m] = h[k - P - m]
            if (k + P - m) in h:
                BH[k, m] = h[k + P - m]
    return B0, BL, BH


def _tile(vec):
    """[4096] -> [128, 32], i = k + 128*c."""
    return np.ascontiguousarray(np.asarray(vec).reshape(CCOL, P).T)


def host_constants(target, A, x0):
    """All f64 precompute; returns the [128, BLOB_COLS] f32 device blob."""
    A64 = np.asarray(A, np.float64)
    w = ALPHA + 2 * GAMMA * (1 - np.cos(2 * np.pi * np.arange(N // 2 + 1) / N))

    def C_inv(z):
        return np.fft.irfft(np.fft.rfft(z, axis=-1) / w, n=N, axis=-1)

    U = C_inv(A64).T                              # [N, 9]
    S = np.eye(R9) + A64 @ U
    W2 = U @ np.linalg.inv(S)                     # [N, 9]
    g = np.fft.irfft(1.0 / w, n=N)                # kernel of C^-1
    b = A64 @ np.asarray(target, np.float64)
    bA = b @ A64
    c0 = C_inv(bA) - W2 @ (U.T @ bA)              # B^-1 @ bA

    blob = np.zeros((P, BLOB_COLS), np.float32)

    def putb(off, arr):
        p = _pack_bf16(arr)
        blob[:p.shape[0], off:off + p.shape[1]] = p

    mA1 = _banded({-1: GAMMA, 0: -GAMMA})         # gamma*(S+ - I)
    mB = _banded({1: GAMMA, 0: -GAMMA})           # gamma*(S- - I)
    mG = _banded({d: g[d % N] for d in range(-RB, RB + 1)})
    Ghi = [_bf16(m).astype(np.float64) for m in mG]
    Glo = [m - h for m, h in zip(mG, Ghi)]

    putb(OFF_A1M, mA1[0]); putb(OFF_A1C, mA1[1])
    putb(OFF_BM, mB[0]); putb(OFF_BC, mB[2])
    putb(OFF_ABM, mA1[0] + mB[0])
    putb(OFF_GHIM, Ghi[0]); putb(OFF_GLOM, Glo[0])
    putb(OFF_GBLH, Ghi[1])
    putb(OFF_GBHH, Ghi[2])
    putb(OFF_ONES, np.ones((P, P)))

    # U2[k, m*32+c] = U[k + 128c, m] (m-major)
    putb(OFF_U2, U.reshape(CCOL, P, R9).transpose(1, 2, 0).reshape(P, R9 * CCOL))
    # W2e[k, c*10+m] = W2[k + 128c, m] for m<9;  c0[k + 128c] at m=9.
    W2e = np.concatenate([W2, c0[:, None]], axis=1)
    putb(OFF_W2, W2e.reshape(CCOL, P, R10).transpose(1, 0, 2).reshape(P, CCOL * R10))

    putb(OFF_X0, _tile(np.asarray(x0, np.float64)))

    # scan reset masks: 0 at each group's first element, 1 elsewhere
    m288 = np.ones((P, R9 * CCOL)); m288[:, 0::CCOL] = 0.0   # groups of 32 (c)
    m320 = np.ones((P, CCOL * R10)); m320[:, 0::R10] = 0.0   # groups of 10 (m)
    putb(OFF_M288, m288)
    putb(OFF_M320, m320)
    neg = np.zeros((P, 2)); neg[:, 0] = -1.0 / P
    putb(OFF_NEG, neg)
    return np.ascontiguousarray(blob)


def build_nc():
    """Build and compile the Bacc graph (one core's program)."""
    from concourse import bacc, mybir, tile

    f32 = mybir.dt.float32
    bf16 = mybir.dt.bfloat16
    Alu = mybir.AluOpType
    nc = bacc.Bacc(target_bir_lowering=False)

    blob_ext = nc.declare_dram_parameter("blob", [P, BLOB_COLS], f32, isOutput=False)
    out_ext = nc.declare_dram_parameter("out", [P, CCOL], f32, isOutput=True)

    with tile.TileContext(nc) as tc:
        with (
            tc.tile_pool(name="const", bufs=1) as cpool,
            tc.tile_pool(name="work", bufs=3) as wpool,
            tc.tile_pool(name="psum", bufs=1, space="PSUM") as ppool,
        ):
            cb = cpool.tile([P, BLOB_COLS], f32, tag="blob")
            nc.sync.dma_start(cb[:, :], blob_ext[:, :])

            def csb(off, wcols):
                """bf16 view of wcols f32 columns -> [128, 2*wcols] bf16"""
                return cb[:, off:off + wcols].bitcast(bf16)

            A1_M, A1_C = csb(OFF_A1M, 64), csb(OFF_A1C, 64)
            B_M, B_C = csb(OFF_BM, 64), csb(OFF_BC, 64)
            AB_M = csb(OFF_ABM, 64)
            GHI_M, GLO_M = csb(OFF_GHIM, 64), csb(OFF_GLOM, 64)
            GBL_H = csb(OFF_GBLH, 64)
            GBH_H = csb(OFF_GBHH, 64)
            ones_bf = csb(OFF_ONES, 64)
            U2 = csb(OFF_U2, 144)                  # [128, 288] bf16
            U2_3d = U2.rearrange("k (m c) -> k m c", c=CCOL)
            W2e = csb(OFF_W2, 160)                 # [128, 320] bf16
            W2_3d = W2e.rearrange("k (c m) -> k c m", m=R10)
            M288 = csb(OFF_M288, 144)
            M320 = csb(OFF_M320, 160)
            NEGC = csb(OFF_NEG, 1)                 # [128, 2]; col 0 = -1/128

            def corner(bank, lhsT, src, shift, stop=False, start=False):
                """Cross-column corner of a banded circulant: out col c
                reads src col c+shift (mod 32), as two payload matmuls."""
                if shift == -1:
                    nc.tensor.matmul(bank[:, 1:CCOL], lhsT, src[:, 0:CCOL - 1],
                                     start=start, stop=False,
                                     skip_group_check=True)
                    nc.tensor.matmul(bank[:, 0:1], lhsT, src[:, CCOL - 1:CCOL],
                                     start=start, stop=stop,
                                     skip_group_check=True)
                else:
                    nc.tensor.matmul(bank[:, 0:CCOL - 1], lhsT, src[:, 1:CCOL],
                                     start=start, stop=False,
                                     skip_group_check=True)
                    nc.tensor.matmul(bank[:, CCOL - 1:CCOL], lhsT, src[:, 0:1],
                                     start=start, stop=stop,
                                     skip_group_check=True)

            Xh = csb(OFF_X0, 16)                   # [128, 32] bf16 state
            bankR = ppool.tile([P, 16], f32, tag="R")

            # persistent seeded PSUM banks (parity-alternating)
            def at_bank(j):
                return ppool.tile([P, CCOL], f32, tag=f"At{j % 2}",
                                  name=f"At{j % 2}")
            def b_bank(j):
                return ppool.tile([P, CCOL], f32, tag=f"B{j % 2}",
                                  name=f"B{j % 2}")

            bankAt = bankB = None   # banks for the current iteration
            T3 = None               # tau-side state (= tTW of next iter)

            for j in range(1, NIT + 1):
                first = (j == 1)
                last = (j == NIT)
                xm = Xh[:, 0:CCOL]

                # --- PE: bankAt (+ seeded tAB), bankB (+ seeded tAB) ---
                if first:
                    bankAt = at_bank(1)
                    nc.tensor.matmul(bankAt[:, :], A1_M, xm, start=True,
                                     stop=False)
                    corner(bankAt, A1_C, xm, -1, stop=True)
                else:
                    nc.tensor.matmul(bankAt[:, :], AB_M, xm, start=False,
                                     stop=False, skip_group_check=True)
                    corner(bankAt, A1_C, xm, -1)
                    corner(bankAt, B_C, xm, +1, stop=True)
                    nc.tensor.matmul(bankB[:, :], B_M, xm, start=False,
                                     stop=False, skip_group_check=True)
                    corner(bankB, B_C, xm, +1, stop=True)

                # --- GpSimd: tau-side chain (fully off the DVE) ---
                # T3' = u3 - relu(u3 + a*x) = min(u3, -a*x), u3 = tau = a*x+tTW
                cxn = wpool.tile([P, CCOL], f32, tag="cxn")
                u3t = wpool.tile([P, CCOL], f32, tag="u3")
                t3n = wpool.tile([P, CCOL], f32, tag=f"t3{j % 2}")
                nc.gpsimd.tensor_scalar_mul(cxn[:, :], xm, -ALPHA)
                if first:
                    nc.vector.tensor_scalar_min(t3n[:, :], cxn[:, :], 0.0)
                else:
                    # u3 = tTW - (-a*x);  T3' = min(u3, -a*x) [min on DVE]
                    nc.gpsimd.tensor_sub(u3t[:, :], T3[:, :], cxn[:, :])
                    nc.vector.tensor_tensor(t3n[:, :], u3t[:, :], cxn[:, :],
                                            Alu.min)

                # --- DVE: soft-threshold, state seed, v ---
                r1 = wpool.tile([P, CCOL], f32, tag="r1")
                Un = wpool.tile([P, CCOL], f32, tag="Un")
                vh = wpool.tile([P, CCOL], bf16, tag="vh")
                vm = vh[:, 0:CCOL]
                nc.vector.tensor_scalar(r1[:, :], bankAt[:, :], -LAM, LAM,
                                        Alu.max, Alu.min)
                nc.vector.tensor_sub(Un[:, :], bankAt[:, :], r1[:, :])
                if last:
                    # no next-iteration banks to seed
                    t2 = wpool.tile([P, CCOL], f32, tag="t2")
                    nc.vector.tensor_sub(t2[:, :], Un[:, :], bankB[:, :])
                    nc.vector.tensor_sub(vm, t2[:, :], t3n[:, :])
                else:
                    at_n = at_bank(j + 1)
                    b_n = b_bank(j + 1)
                    if first:
                        # tAB' = eta_0 - Un = -Un
                        nc.vector.tensor_scalar_mul(at_n[:, :], Un[:, :], -1.0)
                    else:
                        # tAB' = eta - Un = bankB - Un
                        nc.vector.tensor_sub(at_n[:, :], bankB[:, :], Un[:, :])
                    # v = -T3 - tAB'
                    nc.vector.scalar_tensor_tensor(vm, t3n[:, :], -1.0,
                                                   at_n[:, :], Alu.mult,
                                                   Alu.subtract)

                # --- PE: banded G apply (hi+lo mains, hi-only corners) ---
                bankE = ppool.tile([P, CCOL], f32, tag="E")
                nc.tensor.matmul(bankE[:, :], GHI_M, vm, start=True, stop=False)
                nc.tensor.matmul(bankE[:, :], GLO_M, vm, start=False, stop=False)
                corner(bankE, GBL_H, vm, -1)
                corner(bankE, GBH_H, vm, +1, stop=True)

                # --- DVE: rank-9 head  Z1 = U2 (.) v ; segmented scan ---
                Z1 = wpool.tile([P, R9 * CCOL], bf16, tag="Z1")
                z1_3d = Z1[:, :].rearrange("k (m c) -> k m c", c=CCOL)
                vb9 = vm.unsqueeze(1).broadcast_to([P, R9, CCOL])
                nc.vector.tensor_mul(z1_3d, U2_3d, vb9)
                if USE_SCAN:
                    Z1S = wpool.tile([P, R9 * CCOL], bf16, tag="Z1S")
                    nc.vector.tensor_tensor_scan(Z1S[:, :], M288[:, 0:R9 * CCOL],
                                                 Z1[:, :], 0.0,
                                                 Alu.mult, Alu.add)
                    z1tails = Z1S[:, :].rearrange(
                        "k (m c) -> k m c", c=CCOL)[:, :, CCOL - 1:CCOL]
                else:
                    Z1r = wpool.tile([P, 16], bf16, tag="Z1r")
                    with nc.allow_low_precision(reason="q is consumed in f32"):
                        nc.vector.tensor_reduce(Z1r[:, 0:R9], z1_3d,
                                                axis=mybir.AxisListType.X,
                                                op=Alu.add)
                    z1tails = Z1r[:, 0:R9]

                # --- seed copy for the next iteration's bankB (off-path) ---
                if not (first or last):
                    nc.vector.tensor_copy(b_n[:, :], at_n[:, :])
                elif first:
                    nc.vector.tensor_copy(b_n[:, :], at_n[:, :])

                # --- PE: partition reduce + broadcast of q (PSUM f32) ---
                nc.tensor.matmul(bankR[:, 0:R9], ones_bf, z1tails,
                                 start=True, stop=True, skip_group_check=True)
                if first:
                    # c0 slot: q10 = -1 written once; PSUM persists after reads
                    nc.tensor.matmul(bankR[:, R9:R10], ones_bf, NEGC[:, 0:1],
                                     start=True, stop=True,
                                     skip_group_check=True)

                # --- DVE: corr = W2e q (incl. -c0) and the x update ---
                Z2 = wpool.tile([P, CCOL * R10], bf16, tag="Z2")
                z2_3d = Z2[:, :].rearrange("k (c m) -> k c m", m=R10)
                rb = bankR[:, 0:R10].unsqueeze(1).broadcast_to([P, CCOL, R10])
                nc.vector.tensor_mul(z2_3d, W2_3d, rb)
                if USE_SCAN:
                    Z2S = wpool.tile([P, CCOL * R10], bf16, tag="Z2S")
                    nc.vector.tensor_tensor_scan(Z2S[:, :], M320[:, 0:CCOL * R10],
                                                 Z2[:, :], 0.0,
                                                 Alu.mult, Alu.add)
                    corr3 = Z2S[:, :].rearrange(
                        "k (c m) -> k c m", m=R10)[:, :, R10 - 1:R10]
                else:
                    corrt = wpool.tile([P, CCOL], bf16, tag="corr")
                    with nc.allow_low_precision(reason="x state is bf16"):
                        nc.vector.tensor_reduce(corrt[:, :], z2_3d,
                                                axis=mybir.AxisListType.X,
                                                op=Alu.add)
                    corr3 = corrt[:, :].unsqueeze(2)

                bankE3 = bankE[:, :].unsqueeze(2)
                if not last:
                    Xn = wpool.tile([P, CCOL], bf16, tag="Xh")
                    nc.vector.tensor_sub(Xn[:, :].unsqueeze(2), bankE3, corr3)
                    Xh = Xn
                    bankAt, bankB, T3 = at_n, b_n, t3n
                else:
                    Xout = wpool.tile([P, CCOL], f32, tag="Xout")
                    nc.vector.tensor_sub(Xout[:, :].unsqueeze(2), bankE3, corr3)
                    nc.sync.dma_start(out_ext[:, :], Xout[:, :])

    nc.compile()
    return nc


def kernel(**inputs):
    from concourse.bass_utils import run_bass_kernel_spmd

    target = np.asarray(inputs["target"], np.float32)
    A = np.asarray(inputs["A"], np.float32)
    x0 = np.asarray(inputs["x0"], np.float32)

    blob = host_constants(target, A, x0)
    nc = build_nc()
    in_maps = [{"blob": blob} for _ in range(NCORES)]
    res = run_bass_kernel_spmd(nc, in_maps, core_ids=list(range(NCORES)))
    out_tile = np.asarray(res.results[0]["out"], np.float32)
    return np.ascontiguousarray(out_tile.T.reshape(-1))


# revision 9
# speedup vs baseline: 1.3004x; 1.3004x over previous
"""Trainium2 Bass kernel for the ADMM total-variation solver (nn_ADMM).

Math: x <- B^-1(bA + v) iterated 50x, B = AtA + g*DtD + a*I.  AtA is
rank-9 and C := g*DtD + a*I is circulant, so by Woodbury
    B^-1 = C^-1 - W2 U^T,   U = C^-1 A^T,  W2 = U S^-1,  S = I9 + A U.
C^-1 is applied as a banded (radius-32) circular convolution G; the
rank-9 correction uses q = U^T v.  All 8 cores run the same program
(SPMD, no collectives); core 0's output is returned.

v3 design (vs the En/P1/P2n baseline):
- state algebra: tAB := eta - Bx (f32), tTW := tau - alpha*x (f32).
  bankAt = AB x + tAB, bankB = B x + tAB (= eta), tAB' = bankB - Un,
  tTW' = u3 - Wn =: T3, v = -T3 - tAB'.
- tAB' is written by DVE DIRECTLY into the two PSUM banks that the next
  iteration's PE matmuls accumulate onto (start=False) - no I-fold
  matmuls, no bf16 round of the eta state, no En add.
- the tau-side chain (u3 = a*x + tTW, Dt = a*x + u3, Wn = relu Dt,
  T3 = u3 - Wn) runs entirely on GpSimd; the Scalar engine is unused.
- both grouped rank-9 reductions are fused mul+segmented-scan:
  tensor_tensor_scan(state = mask*state + prod) leaves each group's
  total at its last element; the ones-matmul / Xn read the tails via
  strided APs.  q stays in PSUM f32 (no bf16 cast round-trip).
- G = hi+lo bf16 split for the main band; corner (cross-column) pieces
  are hi-only (lo corners dropped; ~2e-4 operator effect).

Vector layout: [128, 32] tiles, flat index i = k + 128*c at tile col c.
"""

import numpy as np

N = 4096
P = 128          # partitions
CCOL = 32        # payload columns; i = k + 128*c
RB = 32          # band radius of G
R9 = 9           # Woodbury rank
R10 = 10         # rank columns incl. the c0 slot
GAMMA = 10.0
ALPHA = 5.0
LAM = 1e-4
NIT = 50
NCORES = 8
USE_SCAN = False  # fused mul+segmented-scan reduces (False: tensor_reduce)
                  # (measured: HW runs tensor_tensor_scan at ~2.6 cyc/elem —
                  # slower than the mode-less grouped reduce; keep reduces)

# f32-column offsets inside the constant blob [128, BLOB_COLS].
# bf16 payloads are packed two-per-f32-column and bitcast on device.
_cur = 0
def _alloc(w):
    global _cur
    off = _cur
    _cur += w
    return off

OFF_A1M   = _alloc(64)    # gamma*(S+ - I) main, bf16 [128,128] (iter 1)
OFF_BM    = _alloc(64)    # gamma*(S- - I) main
OFF_ABM   = _alloc(64)    # gamma*(S+ + S- - 2I) main (A1_M + B_M)
OFF_A1C   = _alloc(64)    # A1 corner (single element, padded)
OFF_BC    = _alloc(64)    # B corner
OFF_GHIM  = _alloc(64)    # C^-1 banded main, hi half
OFF_GLOM  = _alloc(64)    # lo half
OFF_GBLH  = _alloc(64)    # C^-1 left corner hi
OFF_GBHH  = _alloc(64)    # C^-1 right corner hi
OFF_ONES  = _alloc(64)    # all-ones bf16 (partition reduce + broadcast)
OFF_U2    = _alloc(144)   # U m-major: [k, m*32+c], bf16 [128, 288]
OFF_W2    = _alloc(160)   # [W2 | c0] c-major: [k, c*10+m], bf16
OFF_X0    = _alloc(16)    # x0 tile, bf16 [128, 32]
OFF_M288  = _alloc(144)   # scan reset mask for Z1: 0 at c==0, else 1
OFF_M320  = _alloc(160)   # scan reset mask for Z2: 0 at m==0, else 1
OFF_NEG   = _alloc(1)     # [-1/128, 0] bf16 pair (c0 slot of bankR)
BLOB_COLS = _cur


def _bf16(x):
    x32 = np.asarray(x, np.float32)
    u = x32.view(np.uint32)
    r = ((u >> 16) + ((u >> 15) & 1)).astype(np.uint32) << 16
    return r.view(np.float32)


def _pack_bf16(arr):
    """[128, W] float (W even) -> [128, W//2] f32 with packed bf16 pairs."""
    a = _bf16(arr).view(np.uint32) >> 16
    lo, hi = a[:, 0::2], a[:, 1::2]
    return (lo | (hi << 16)).view(np.float32)


def _banded(h):
    """main/BL/BH lhsT pieces for kernel h (dict d -> coef), [128,128] each.
    lhsT[k, m]: contraction index k = input row, m = output row.
    main: within-column (shift d = k - m);
    BL: rhs = col c-1 view (shift d = k - 128 - m);
    BH: rhs = col c+1 view (shift d = k + 128 - m)."""
    B0 = np.zeros((P, P)); BL = np.zeros((P, P)); BH = np.zeros((P, P))
    for k in range(P):
        for m in range(P):
            if (k - m) in h:
                B0[k, m] = h[k - m]
            if (k - P - m) in h:
                BL[k, m] = h[k - P - m]
            if (k + P - m) in h:
                BH[k, m] = h[k + P - m]
    return B0, BL, BH


def _tile(vec):
    """[4096] -> [128, 32], i = k + 128*c."""
    return np.ascontiguousarray(np.asarray(vec).reshape(CCOL, P).T)


def host_constants(target, A, x0):
    """All f64 precompute; returns the [128, BLOB_COLS] f32 device blob."""
    A64 = np.asarray(A, np.float64)
    w = ALPHA + 2 * GAMMA * (1 - np.cos(2 * np.pi * np.arange(N // 2 + 1) / N))

    def C_inv(z):
        return np.fft.irfft(np.fft.rfft(z, axis=-1) / w, n=N, axis=-1)

    U = C_inv(A64).T                              # [N, 9]
    S = np.eye(R9) + A64 @ U
    W2 = U @ np.linalg.inv(S)                     # [N, 9]
    g = np.fft.irfft(1.0 / w, n=N)                # kernel of C^-1
    b = A64 @ np.asarray(target, np.float64)
    bA = b @ A64
    c0 = C_inv(bA) - W2 @ (U.T @ bA)              # B^-1 @ bA

    blob = np.zeros((P, BLOB_COLS), np.float32)

    def putb(off, arr):
        p = _pack_bf16(arr)
        blob[:p.shape[0], off:off + p.shape[1]] = p

    mA1 = _banded({-1: GAMMA, 0: -GAMMA})         # gamma*(S+ - I)
    mB = _banded({1: GAMMA, 0: -GAMMA})           # gamma*(S- - I)
    mG = _banded({d: g[d % N] for d in range(-RB, RB + 1)})
    Ghi = [_bf16(m).astype(np.float64) for m in mG]
    Glo = [m - h for m, h in zip(mG, Ghi)]

    putb(OFF_A1M, mA1[0]); putb(OFF_A1C, mA1[1])
    putb(OFF_BM, mB[0]); putb(OFF_BC, mB[2])
    putb(OFF_ABM, mA1[0] + mB[0])
    putb(OFF_GHIM, Ghi[0]); putb(OFF_GLOM, Glo[0])
    putb(OFF_GBLH, Ghi[1])
    putb(OFF_GBHH, Ghi[2])
    putb(OFF_ONES, np.ones((P, P)))

    # U2[k, m*32+c] = U[k + 128c, m] (m-major)
    putb(OFF_U2, U.reshape(CCOL, P, R9).transpose(1, 2, 0).reshape(P, R9 * CCOL))
    # W2e[k, c*10+m] = W2[k + 128c, m] for m<9;  c0[k + 128c] at m=9.
    W2e = np.concatenate([W2, c0[:, None]], axis=1)
    putb(OFF_W2, W2e.reshape(CCOL, P, R10).transpose(1, 0, 2).reshape(P, CCOL * R10))

    putb(OFF_X0, _tile(np.asarray(x0, np.float64)))

    # scan reset masks: 0 at each group's first element, 1 elsewhere
    m288 = np.ones((P, R9 * CCOL)); m288[:, 0::CCOL] = 0.0   # groups of 32 (c)
    m320 = np.ones((P, CCOL * R10)); m320[:, 0::R10] = 0.0   # groups of 10 (m)
    putb(OFF_M288, m288)
    putb(OFF_M320, m320)
    neg = np.zeros((P, 2)); neg[:, 0] = -1.0 / P
    putb(OFF_NEG, neg)
    return np.ascontiguousarray(blob)


def build_nc():
    """Build and compile the Bacc graph (one core's program)."""
    from concourse import bacc, mybir, tile

    f32 = mybir.dt.float32
    bf16 = mybir.dt.bfloat16
    Alu = mybir.AluOpType
    nc = bacc.Bacc(target_bir_lowering=False)

    blob_ext = nc.declare_dram_parameter("blob", [P, BLOB_COLS], f32, isOutput=False)
    out_ext = nc.declare_dram_parameter("out", [P, CCOL], f32, isOutput=True)

    with tile.TileContext(nc) as tc:
        with (
            tc.tile_pool(name="const", bufs=1) as cpool,
            tc.tile_pool(name="work", bufs=3) as wpool,
            tc.tile_pool(name="psum", bufs=1, space="PSUM") as ppool,
        ):
            cb = cpool.tile([P, BLOB_COLS], f32, tag="blob")
            nc.sync.dma_start(cb[:, :], blob_ext[:, :])

            def csb(off, wcols):
                """bf16 view of wcols f32 columns -> [128, 2*wcols] bf16"""
                return cb[:, off:off + wcols].bitcast(bf16)

            A1_M, A1_C = csb(OFF_A1M, 64), csb(OFF_A1C, 64)
            B_M, B_C = csb(OFF_BM, 64), csb(OFF_BC, 64)
            AB_M = csb(OFF_ABM, 64)
            GHI_M, GLO_M = csb(OFF_GHIM, 64), csb(OFF_GLOM, 64)
            GBL_H = csb(OFF_GBLH, 64)
            GBH_H = csb(OFF_GBHH, 64)
            ones_bf = csb(OFF_ONES, 64)
            U2 = csb(OFF_U2, 144)                  # [128, 288] bf16
            U2_3d = U2.rearrange("k (m c) -> k m c", c=CCOL)
            W2e = csb(OFF_W2, 160)                 # [128, 320] bf16
            W2_3d = W2e.rearrange("k (c m) -> k c m", m=R10)
            M288 = csb(OFF_M288, 144)
            M320 = csb(OFF_M320, 160)
            NEGC = csb(OFF_NEG, 1)                 # [128, 2]; col 0 = -1/128

            def corner(bank, lhsT, src, shift, stop=False, start=False):
                """Cross-column corner of a banded circulant: out col c
                reads src col c+shift (mod 32), as two payload matmuls."""
                if shift == -1:
                    nc.tensor.matmul(bank[:, 1:CCOL], lhsT, src[:, 0:CCOL - 1],
                                     start=start, stop=False,
                                     skip_group_check=True)
                    nc.tensor.matmul(bank[:, 0:1], lhsT, src[:, CCOL - 1:CCOL],
                                     start=start, stop=stop,
                                     skip_group_check=True)
                else:
                    nc.tensor.matmul(bank[:, 0:CCOL - 1], lhsT, src[:, 1:CCOL],
                                     start=start, stop=False,
                                     skip_group_check=True)
                    nc.tensor.matmul(bank[:, CCOL - 1:CCOL], lhsT, src[:, 0:1],
                                     start=start, stop=stop,
                                     skip_group_check=True)

            Xh = csb(OFF_X0, 16)                   # [128, 32] bf16 state
            bankR = ppool.tile([P, 16], f32, tag="R")

            # persistent seeded PSUM banks (parity-alternating)
            def at_bank(j):
                return ppool.tile([P, CCOL], f32, tag=f"At{j % 2}",
                                  name=f"At{j % 2}")
            def b_bank(j):
                return ppool.tile([P, CCOL], f32, tag=f"B{j % 2}",
                                  name=f"B{j % 2}")

            bankAt = bankB = None   # banks for the current iteration
            T3 = None               # tau-side state (= tTW of next iter)

            for j in range(1, NIT + 1):
                first = (j == 1)
                last = (j == NIT)
                xm = Xh[:, 0:CCOL]

                # --- PE: bankAt (+ seeded tAB), bankB (+ seeded tAB) ---
                if first:
                    bankAt = at_bank(1)
                    nc.tensor.matmul(bankAt[:, :], A1_M, xm, start=True,
                                     stop=False)
                    corner(bankAt, A1_C, xm, -1, stop=True)
                else:
                    nc.tensor.matmul(bankAt[:, :], AB_M, xm, start=False,
                                     stop=False, skip_group_check=True)
                    corner(bankAt, A1_C, xm, -1)
                    corner(bankAt, B_C, xm, +1, stop=True)
                    nc.tensor.matmul(bankB[:, :], B_M, xm, start=False,
                                     stop=False, skip_group_check=True)
                    corner(bankB, B_C, xm, +1, stop=True)

                # --- DVE (pre-r1 idle window): tau-side chain ---
                # T3' = u3 - relu(u3 + a*x) = min(u3, -a*x), u3 = tau = a*x+tTW
                cxn = wpool.tile([P, CCOL], f32, tag="cxn")
                u3t = wpool.tile([P, CCOL], f32, tag="u3")
                t3n = wpool.tile([P, CCOL], f32, tag=f"t3{j % 2}")
                nc.vector.tensor_scalar_mul(cxn[:, :], xm, -ALPHA)
                if first:
                    nc.vector.tensor_scalar_min(t3n[:, :], cxn[:, :], 0.0)
                else:
                    # u3 = tTW - (-a*x);  T3' = min(u3, -a*x)
                    nc.vector.tensor_sub(u3t[:, :], T3[:, :], cxn[:, :])
                    nc.vector.tensor_tensor(t3n[:, :], u3t[:, :], cxn[:, :],
                                            Alu.min)

                # --- DVE: soft-threshold, state seed, v ---
                r1 = wpool.tile([P, CCOL], f32, tag="r1")
                Un = wpool.tile([P, CCOL], f32, tag="Un")
                vh = wpool.tile([P, CCOL], bf16, tag="vh")
                vm = vh[:, 0:CCOL]
                nc.vector.tensor_scalar(r1[:, :], bankAt[:, :], -LAM, LAM,
                                        Alu.max, Alu.min)
                nc.vector.tensor_sub(Un[:, :], bankAt[:, :], r1[:, :])
                if last:
                    # no next-iteration banks to seed
                    t2 = wpool.tile([P, CCOL], f32, tag="t2")
                    nc.vector.tensor_sub(t2[:, :], Un[:, :], bankB[:, :])
                    nc.vector.tensor_sub(vm, t2[:, :], t3n[:, :])
                else:
                    at_n = at_bank(j + 1)
                    b_n = b_bank(j + 1)
                    if first:
                        # tAB' = eta_0 - Un = -Un
                        nc.vector.tensor_scalar_mul(at_n[:, :], Un[:, :], -1.0)
                    else:
                        # tAB' = eta - Un = bankB - Un
                        nc.vector.tensor_sub(at_n[:, :], bankB[:, :], Un[:, :])
                    # v = -T3 - tAB'
                    nc.vector.scalar_tensor_tensor(vm, t3n[:, :], -1.0,
                                                   at_n[:, :], Alu.mult,
                                                   Alu.subtract)

                # --- PE: banded G apply (hi+lo mains, hi-only corners) ---
                bankE = ppool.tile([P, CCOL], f32, tag="E")
                nc.tensor.matmul(bankE[:, :], GHI_M, vm, start=True, stop=False)
                nc.tensor.matmul(bankE[:, :], GLO_M, vm, start=False, stop=False)
                corner(bankE, GBL_H, vm, -1)
                corner(bankE, GBH_H, vm, +1, stop=True)

                # --- DVE: rank-9 head  Z1 = U2 (.) v ; segmented scan ---
                Z1 = wpool.tile([P, R9 * CCOL], bf16, tag="Z1")
                z1_3d = Z1[:, :].rearrange("k (m c) -> k m c", c=CCOL)
                vb9 = vm.unsqueeze(1).broadcast_to([P, R9, CCOL])
                nc.vector.tensor_mul(z1_3d, U2_3d, vb9)
                if USE_SCAN:
                    Z1S = wpool.tile([P, R9 * CCOL], bf16, tag="Z1S")
                    nc.vector.tensor_tensor_scan(Z1S[:, :], M288[:, 0:R9 * CCOL],
                                                 Z1[:, :], 0.0,
                                                 Alu.mult, Alu.add)
                    z1tails = Z1S[:, :].rearrange(
                        "k (m c) -> k m c", c=CCOL)[:, :, CCOL - 1:CCOL]
                else:
                    Z1r = wpool.tile([P, 16], bf16, tag="Z1r")
                    with nc.allow_low_precision(reason="q is consumed in f32"):
                        nc.vector.tensor_reduce(Z1r[:, 0:R9], z1_3d,
                                                axis=mybir.AxisListType.X,
                                                op=Alu.add)
                    z1tails = Z1r[:, 0:R9]

                # --- seed copy for the next iteration's bankB (off-path) ---
                if not last:
                    nc.vector.tensor_copy(b_n[:, :], at_n[:, :])

                # --- PE: partition reduce + broadcast of q (PSUM f32) ---
                nc.tensor.matmul(bankR[:, 0:R9], ones_bf, z1tails,
                                 start=True, stop=True, skip_group_check=True)
                if first:
                    # c0 slot: q10 = -1 written once; PSUM persists after reads
                    nc.tensor.matmul(bankR[:, R9:R10], ones_bf, NEGC[:, 0:1],
                                     start=True, stop=True,
                                     skip_group_check=True)

                # --- DVE: corr = W2e q (incl. -c0) and the x update ---
                Z2 = wpool.tile([P, CCOL * R10], bf16, tag="Z2")
                z2_3d = Z2[:, :].rearrange("k (c m) -> k c m", m=R10)
                rb = bankR[:, 0:R10].unsqueeze(1).broadcast_to([P, CCOL, R10])
                nc.vector.tensor_mul(z2_3d, W2_3d, rb)
                if USE_SCAN:
                    Z2S = wpool.tile([P, CCOL * R10], bf16, tag="Z2S")
                    nc.vector.tensor_tensor_scan(Z2S[:, :], M320[:, 0:CCOL * R10],
                                                 Z2[:, :], 0.0,
                                                 Alu.mult, Alu.add)
                    corr3 = Z2S[:, :].rearrange(
                        "k (c m) -> k c m", m=R10)[:, :, R10 - 1:R10]
                else:
                    corrt = wpool.tile([P, CCOL], bf16, tag="corr")
                    with nc.allow_low_precision(reason="x state is bf16"):
                        nc.vector.tensor_reduce(corrt[:, :], z2_3d,
                                                axis=mybir.AxisListType.X,
                                                op=Alu.add)
                    corr3 = corrt[:, :].unsqueeze(2)

                bankE3 = bankE[:, :].unsqueeze(2)
                if not last:
                    Xn = wpool.tile([P, CCOL], bf16, tag="Xh")
                    nc.vector.tensor_sub(Xn[:, :].unsqueeze(2), bankE3, corr3)
                    Xh = Xn
                    bankAt, bankB, T3 = at_n, b_n, t3n
                else:
                    Xout = wpool.tile([P, CCOL], f32, tag="Xout")
                    nc.vector.tensor_sub(Xout[:, :].unsqueeze(2), bankE3, corr3)
                    nc.sync.dma_start(out_ext[:, :], Xout[:, :])

    nc.compile()
    return nc


def kernel(**inputs):
    from concourse.bass_utils import run_bass_kernel_spmd

    target = np.asarray(inputs["target"], np.float32)
    A = np.asarray(inputs["A"], np.float32)
    x0 = np.asarray(inputs["x0"], np.float32)

    blob = host_constants(target, A, x0)
    nc = build_nc()
    in_maps = [{"blob": blob} for _ in range(NCORES)]
    res = run_bass_kernel_spmd(nc, in_maps, core_ids=list(range(NCORES)))
    out_tile = np.asarray(res.results[0]["out"], np.float32)
    return np.ascontiguousarray(out_tile.T.reshape(-1))
